# revision 16
# baseline (speedup 1.0000x reference)
"""Trainium2 Bass kernel for a sparse-attention decoder block.

Reference computation (single core, jax):
  src = concat([x, pos], 1)                      # [S=2048, 136]
  tgt = (src @ proj_w.T + proj_b) -> [5S, 512]
  q/k/v projections, banded multihead attention (band ~27 src cols/query),
  out-proj, layernorm + residual, 3-layer conv1d (k=3, softplus),
  skip proj, final layernorm.  Output [10240, 512].

Sharding: the 5S=10240 query/row dimension is split across 8 cores
(1280 rows each + 3-row halo each side for the conv stack).  The banded
mask means each core only needs a 304-row slice of src.  Everything else
(weights) is replicated.  No collectives; the conv halo is recomputed
locally; out-of-range halo rows at the sequence edges are zeroed on
device via tiny per-core edge masks (conv zero-padding semantics).

Precision: q/k/v projections run in fp32r (full PE rate at N>=256);
attention (scores, probs, attn-out), out-proj, conv stack and skip-proj
run in bf16 with fp32 PSUM accumulation (measured end-to-end rel err
~3e-3, threshold 2e-2).  bf16 doubles effective PE rate on the small
attention matmuls (which pay a 4x fp32r penalty below 256 cols) and
halves transpose cost and weight DMA traffic.
"""

import numpy as np
import ml_dtypes

S = 2048
PROJ = 128
DIM = 512
NPOS = 8
KD = PROJ + NPOS        # 136
HEADS = 4
HD = DIM // HEADS       # 128
EXT = 64
L = 5 * S               # 10240
NL = 3
NC = 8                  # cores

RPC = L // NC           # 1280 rows per core
HALO = 3
R = RPC + 2 * HALO      # 1286 local rows
W = 304                 # src slice width
WIN = 64                # attention window per 128-query tile
NT = 11                 # query tiles per core
TILE_OFF = [128 * t for t in range(10)] + [R - 128]          # 1158 last
WJ0 = [((lt - 71) // 5) + 16 for lt in TILE_OFF]             # window starts
RN = [min(128, R - 128 * rt) for rt in range(NT)]            # rows per tile

BF16 = ml_dtypes.bfloat16


def _pos_embed():
    n = np.arange(S, dtype=np.float64)[:, None]
    e = np.arange(NPOS)
    return ((n % (2.0 ** (e + 1))) / (2.0 ** e)).astype(np.float32)


def host_prep(x, residual, proj_w, proj_b, q_w, k_w, v_w, in_b, out_w, out_b,
              conv_w, conv_b, skip_w, skip_b):
    """Build the per-core input maps (all numpy, cheap)."""
    x = np.asarray(x, np.float32)
    residual = np.asarray(residual, np.float32)
    proj_w = np.asarray(proj_w, np.float32)
    proj_b = np.asarray(proj_b, np.float32)
    q_w = np.asarray(q_w, np.float32)
    k_w = np.asarray(k_w, np.float32)
    v_w = np.asarray(v_w, np.float32)
    in_b = np.asarray(in_b, np.float32)
    out_w = np.asarray(out_w, np.float32)
    out_b = np.asarray(out_b, np.float32)
    conv_w = np.asarray(conv_w, np.float32)
    conv_b = np.asarray(conv_b, np.float32)
    skip_w = np.asarray(skip_w, np.float32)
    skip_b = np.asarray(skip_b, np.float32)
    src = np.concatenate([x, _pos_embed()], axis=1)            # [S, KD]

    scale = 1.0 / np.sqrt(np.float32(HD))
    # fold proj into q:  q_p = src @ (q_w @ proj_w_p).T + qb_p
    qwT = np.empty((KD, 5 * DIM), np.float32)
    qb = np.empty((5, DIM), np.float32)
    for p in range(5):
        blk = proj_w[DIM * p:DIM * (p + 1), :]                 # [512, KD]
        fused = q_w @ blk                                      # [512, KD]
        qwT[:, DIM * p:DIM * (p + 1)] = fused.T * scale
        qb[p] = (q_w @ proj_b[DIM * p:DIM * (p + 1)] + in_b[:DIM]) * scale
    # k bias dropped (softmax shift invariance); v bias folded into out bias
    out_b_eff = out_b + out_w @ in_b[2 * DIM:3 * DIM]
    has_bias = bool(np.any(out_b_eff) or np.any(skip_b))

    kwT = np.ascontiguousarray(k_w.T).astype(BF16)             # [KD, 512]
    vwT = np.ascontiguousarray(v_w.T)
    qwT_bf = qwT.astype(BF16)

    # out/skip weights: [128, 4, 512] bf16 (chunk m = input features
    # 128m..128m+127 on partitions)
    owT_t = np.ascontiguousarray(
        out_w.T.reshape(4, 128, DIM).transpose(1, 0, 2)).astype(BF16)
    swT_t = np.ascontiguousarray(
        skip_w.T.reshape(4, 128, DIM).transpose(1, 0, 2)).astype(BF16)
    # conv weights: per layer [128, (d,k), 512] bf16:
    #   cw[li][p, 4d+k, j] = conv_w[li, j, 128k+p, d]
    cw_t = np.ascontiguousarray(
        conv_w.transpose(0, 3, 2, 1)                           # [li, d, in, out]
        .reshape(NL, 3, 4, 128, DIM).transpose(0, 3, 1, 2, 4)  # [li,p,d,k,out]
        .reshape(NL, 128, 12, DIM)).astype(BF16)

    qb_t = np.ascontiguousarray(
        qb.reshape(5, 4, 128).transpose(2, 0, 1).reshape(128, 20))
    cb_t = np.ascontiguousarray(
        conv_b.reshape(NL, 4, 128).transpose(2, 0, 1).reshape(128, NL * 4))
    ob_r = (out_b_eff.astype(np.float32)).reshape(1, DIM)
    sb_r = (skip_b.astype(np.float32)).reshape(1, DIM)

    in_maps = []
    for c in range(NC):
        i0c = 256 * c - 16
        gl0 = RPC * c - HALO

        # src slice [W, KD] with zero pad outside [0, S)
        sl = np.zeros((W, KD), np.float32)
        lo, hi = max(0, i0c), min(S, i0c + W)
        sl[lo - i0c:hi - i0c] = src[lo:hi]
        srcT = np.ascontiguousarray(sl.T)                      # [KD, W]

        # residual tiles [128, NT, DIM] (tile rt = local rows TILE_OFF[rt]..)
        rs = np.zeros((R, DIM), np.float32)
        rlo, rhi = max(0, gl0), min(L, gl0 + R)
        rs[rlo - gl0:rhi - gl0] = residual[rlo:rhi]
        resid_t = np.ascontiguousarray(
            np.stack([rs[TILE_OFF[t]:TILE_OFF[t] + 128] for t in range(NT)],
                     axis=1)).astype(BF16)                     # [128, NT, DIM]

        # attention masks, head-replicated: [128, NT*4*WIN] bf16
        m = np.zeros((128, NT, HEADS, WIN), np.float32)
        for t in range(NT):
            gl = gl0 + TILE_OFF[t] + np.arange(128)[:, None]   # [128,1]
            gi = i0c + WJ0[t] + np.arange(WIN)[None, :]        # [1,WIN]
            allowed = ((gi >= 0) & (gi < S) &
                       (gl >= 5 * gi - EXT) & (gl < 5 * gi + 5 + EXT))
            care = (gl >= 0) & (gl < L)                        # real rows
            m[:, t, :, :] = np.where(~care | allowed, 0.0, -30000.0)[:, None, :]
        masks4 = np.ascontiguousarray(m.reshape(128, NT * HEADS * WIN)).astype(BF16)

        edgeL = np.ones((128, 4, HALO), np.float32)
        edgeR = np.ones((128, 4, HALO), np.float32)
        if c == 0:
            edgeL[:] = 0.0
        if c == NC - 1:
            edgeR[:] = 0.0

        in_maps.append({
            "srcT": srcT, "srcT_bf": srcT.astype(BF16),
            "resid": resid_t, "qwT": qwT_bf, "qb": qb_t,
            "kwT": kwT, "vwT": vwT, "owT": owT_t, "swT": swT_t,
            "cw": cw_t, "cb": cb_t, "masks": masks4,
            "edgeL": edgeL.reshape(128, 12), "edgeR": edgeR.reshape(128, 12),
            "ob": ob_r, "sb": sb_r,
        })
    return in_maps, has_bias


# ---------------------------------------------------------------- device ---

_CACHE = {}


def _build_bass(has_bias=False):
    import concourse.bass as bass
    import concourse.mybir as mybir
    import concourse.tile as tile
    from concourse import bacc
    from concourse.masks import make_identity
    from contextlib import ExitStack

    f32 = mybir.dt.float32
    f32r = mybir.dt.float32r
    bf16 = mybir.dt.bfloat16
    AF = mybir.ActivationFunctionType
    ALU = mybir.AluOpType

    # All ACT funcs used here (Exp, Ln, Identity, Copy) live in the single
    # table 'natural_log_exp_and_others'.  The table-load pass picks the
    # first table containing each func, which alternates exp_and_others /
    # natural_log and inserts ~1.3us table loads.  Empty every other
    # table's func set (keeping dict order, so act_func_set_id keeps
    # matching walrus's act_info.json) to pin all activations to the
    # shared table -> one load.  (The toolchain's softplus table maps no
    # Softplus func, so softplus is computed as Ln(1 + Exp(x)).)
    import concourse.hw_specs as _hw
    import concourse.bacc as _bacc_mod
    import concourse.bass_interp as _interp_mod
    if not getattr(_hw, "_act_tables_pinned", False):
        _orig_gat = _hw.get_activation_tables

        def _pinned_gat(arch):
            t = _orig_gat(arch)
            return {name: (funcs if name == "natural_log_exp_and_others"
                           else set())
                    for name, funcs in t.items()}

        _hw.get_activation_tables = _pinned_gat
        _bacc_mod.get_activation_tables = _pinned_gat
        _interp_mod.get_activation_tables = _pinned_gat
        _hw._act_tables_pinned = True

    nc = bacc.Bacc()

    def din(name, shape, dt=f32):
        return nc.dram_tensor(name, shape, dt, kind="ExternalInput")

    srcT_d = din("srcT", [KD, W], f32r)
    srcbf_d = din("srcT_bf", [KD, W], bf16)
    resid_d = din("resid", [128, NT, DIM], bf16)
    qwT_d = din("qwT", [KD, 5 * DIM], bf16)
    qb_d = din("qb", [128, 20])
    kwT_d = din("kwT", [KD, DIM], bf16)
    vwT_d = din("vwT", [KD, DIM], f32r)
    owT_d = din("owT", [128, 4, DIM], bf16)
    swT_d = din("swT", [128, 4, DIM], bf16)
    cw_d = din("cw", [NL, 128, 12, DIM], bf16)
    cb_d = din("cb", [128, NL * 4])
    masks_d = din("masks", [128, NT * HEADS * WIN], bf16)
    edgeL_d = din("edgeL", [128, 12])
    edgeR_d = din("edgeR", [128, 12])
    if has_bias:
        ob_d = din("ob", [1, DIM])
        sb_d = din("sb", [1, DIM])
    out_d = nc.dram_tensor("out", [RPC, DIM], f32, kind="ExternalOutput")

    KCH = [(0, 128), (128, KD - 128)]        # contraction chunks over KD

    ctx = ExitStack()
    with tile.TileContext(nc) as tc:
        persist = ctx.enter_context(tc.tile_pool(name="persist", bufs=1))
        stream = ctx.enter_context(tc.tile_pool(name="stream", bufs=3))

        ident_bf = persist.tile([128, 128], bf16)
        make_identity(nc, ident_bf)
        ident_f = persist.tile([128, 128], f32)
        make_identity(nc, ident_f)
        eps_t = persist.tile([128, 1], f32)
        nc.vector.memset(eps_t, 1e-5)

        # ---- SBUF tensors (phase-scoped pools to fit 208KB) --------------
        attnp_cm = tc.tile_pool(name="attnp", bufs=1)
        attnp = attnp_cm.__enter__()
        qkvw_cm = tc.tile_pool(name="qkvw", bufs=1)
        qkvw = qkvw_cm.__enter__()

        srcT = [qkvw.tile([kn, W], f32r, name=f"srcT{ki}")
                for ki, (k0, kn) in enumerate(KCH)]
        srcB = [qkvw.tile([kn, W], bf16, name=f"srcB{ki}")
                for ki, (k0, kn) in enumerate(KCH)]
        qwT = [qkvw.tile([kn, 5 * DIM], bf16, name=f"qwT{ki}")
               for ki, (k0, kn) in enumerate(KCH)]
        kwT = [qkvw.tile([kn, DIM], bf16, name=f"kwT{ki}")
               for ki, (k0, kn) in enumerate(KCH)]
        vwT = [qkvw.tile([kn, DIM], f32r, name=f"vwT{ki}")
               for ki, (k0, kn) in enumerate(KCH)]
        masks = attnp.tile([128, NT * HEADS * WIN], bf16)
        qb_t = persist.tile([128, 20], f32)
        resid_t = persist.tile([128, NT, DIM], bf16)
        owT = persist.tile([128, 4, DIM], bf16)
        swT = persist.tile([128, 4, DIM], bf16)
        cw = [persist.tile([128, 12, DIM], bf16, name=f"cw{li}")
              for li in range(NL)]
        cb_t = persist.tile([128, NL * 4], f32)
        edgeL = persist.tile([128, 4, HALO], f32)
        edgeR = persist.tile([128, 4, HALO], f32)

        qT = [attnp.tile([128, R + 4], bf16, name=f"qT{m}") for m in range(4)]
        kT = [attnp.tile([128, W], bf16, name=f"kT{m}") for m in range(4)]
        v_win = attnp.tile([WIN, NT, DIM], bf16)
        oT = attnp.tile([128, HEADS, R], bf16)
        cnnT = persist.tile([128, 4, R + 2], bf16)
        h1 = persist.tile([128, 4, R + 2], bf16)
        h2 = persist.tile([128, 4, R + 2], bf16)
        h3 = persist.tile([128, 4, R + 2], f32)

        # ---- input DMAs ---------------------------------------------------
        # The sim's DMA engine is serial and the two HWDGE queues (SP/ACT)
        # round-robin, so order by first-use time across both queues.
        for ki, (k0, kn) in enumerate(KCH):
            nc.sync.dma_start(out=srcT[ki], in_=srcT_d[k0:k0 + kn, :])
        for ki, (k0, kn) in enumerate(KCH):
            nc.scalar.dma_start(out=srcB[ki], in_=srcbf_d[k0:k0 + kn, :])
        for ki, (k0, kn) in enumerate(KCH):
            nc.sync.dma_start(out=kwT[ki], in_=kwT_d[k0:k0 + kn, :])
        for ki, (k0, kn) in enumerate(KCH):
            nc.scalar.dma_start(out=vwT[ki], in_=vwT_d[k0:k0 + kn, :])
        nc.sync.dma_start(out=qb_t, in_=qb_d[:, :])
        for ki, (k0, kn) in enumerate(KCH):
            nc.scalar.dma_start(out=qwT[ki], in_=qwT_d[k0:k0 + kn, :])
        nc.sync.dma_start(out=masks, in_=masks_d[:, :])
        nc.scalar.dma_start(out=owT, in_=owT_d[:, :, :])
        nc.sync.dma_start(out=resid_t[:, 0:3, :], in_=resid_d[:, 0:3, :])
        nc.scalar.dma_start(out=resid_t[:, 3:NT, :], in_=resid_d[:, 3:NT, :])
        nc.sync.dma_start(out=cw[0], in_=cw_d[0, :, :, :])
        nc.scalar.dma_start(out=cw[1], in_=cw_d[1, :, :, :])
        nc.sync.dma_start(out=cw[2], in_=cw_d[2, :, :, :])
        nc.scalar.dma_start(out=swT, in_=swT_d[:, :, :])
        nc.sync.dma_start(out=cb_t, in_=cb_d[:, :])
        nc.scalar.dma_start(out=edgeL, in_=edgeL_d[:, :])
        nc.scalar.dma_start(out=edgeR, in_=edgeR_d[:, :])
        if has_bias:
            ob_r = persist.tile([1, DIM], f32r)
            sb_r = persist.tile([1, DIM], f32r)
            ones_r = persist.tile([1, 128], f32r)
            nc.scalar.dma_start(out=ob_r, in_=ob_d[:, :])
            nc.scalar.dma_start(out=sb_r, in_=sb_d[:, :])
            nc.vector.memset(ones_r, 1.0)

        # zero the conv pad columns (cols 0 and R+1)
        for tl in (cnnT, h1, h2, h3):
            nc.gpsimd.memset(tl[:, :, 0:1], 0.0)
            nc.gpsimd.memset(tl[:, :, R + 1:R + 2], 0.0)

        # ---- q/k/v projections (fp32r) -----------------------------------
        psP_cm = tc.tile_pool(name="psP", bufs=3, space="PSUM")
        psP = psP_cm.__enter__()
        for m in range(4):
            ps = psP.tile([128, W], f32, tag="proj", name="ps_kproj")
            for ki, (k0, kn) in enumerate(KCH):
                nc.tensor.matmul(ps, kwT[ki][:, 128 * m:128 * (m + 1)],
                                 srcB[ki][:, :],
                                 start=(ki == 0), stop=(ki == len(KCH) - 1))
            nc.scalar.activation(out=kT[m], in_=ps, func=AF.Copy, scale=1.0)

        for t in range(NT):
            ps = psP.tile([WIN, DIM], f32, tag="proj", name="ps_vproj")
            for ki, (k0, kn) in enumerate(KCH):
                nc.tensor.matmul(ps, srcT[ki][:, WJ0[t]:WJ0[t] + WIN],
                                 vwT[ki][:, :],
                                 start=(ki == 0), stop=(ki == len(KCH) - 1))
            nc.scalar.activation(out=v_win[:, t, :], in_=ps, func=AF.Copy,
                                 scale=1.0)

        for p in range(5):
            off = (p + 3) % 5
            cnt = 258                      # padded even (fp32r ISA rule)
            s0 = 16 if p < 2 else 15
            for m in range(4):
                ps = psP.tile([128, cnt], f32, tag="proj", name="ps_qproj")
                for ki, (k0, kn) in enumerate(KCH):
                    nc.tensor.matmul(
                        ps, qwT[ki][:, DIM * p + 128 * m:DIM * p + 128 * (m + 1)],
                        srcB[ki][:, s0:s0 + cnt],
                        start=(ki == 0), stop=(ki == len(KCH) - 1))
                nc.gpsimd.tensor_scalar_add(
                    qT[m][:, off:off + 5 * (cnt - 1) + 1:5], ps,
                    qb_t[:, 4 * p + m:4 * p + m + 1])
        psP_cm.__exit__(None, None, None)
        qkvw_cm.__exit__(None, None, None)

        # ---- attention + out-proj + LN1, interleaved per tile ------------
        psA_cm = tc.tile_pool(name="psA", bufs=2, space="PSUM")
        psA = psA_cm.__enter__()
        psB_cm = tc.tile_pool(name="psB", bufs=2, space="PSUM")
        psB = psB_cm.__enter__()
        for t in range(NT):
            lt = TILE_OFF[t]
            wj = WJ0[t]
            ps_s = psA.tile([128, HEADS * WIN], f32, tag="scores",
                            name="ps_s", bufs=2)
            for h in range(HEADS):
                nc.tensor.matmul(ps_s[:, WIN * h:WIN * (h + 1)],
                                 qT[h][:, lt:lt + 128],
                                 kT[h][:, wj:wj + WIN], start=True, stop=True)
            nc.gpsimd.tensor_add(ps_s, ps_s,
                                 masks[:, 256 * t:256 * (t + 1)])
            probs = stream.tile([128, HEADS, WIN], bf16, tag="probs", bufs=8)
            nc.scalar.activation(out=probs, in_=ps_s, func=AF.Exp)
            sums4 = stream.tile([128, HEADS], f32, tag="sums4", bufs=8)
            nc.vector.tensor_reduce(sums4, probs, axis=mybir.AxisListType.X,
                                    op=ALU.add)
            rs4 = stream.tile([128, HEADS], f32, tag="rs4", bufs=8)
            nc.vector.reciprocal(rs4, sums4)
            pn = stream.tile([128, HEADS, WIN], bf16, tag="pn", bufs=8)
            for h in range(HEADS):
                nc.vector.tensor_scalar_mul(pn[:, h, :], probs[:, h, :],
                                            rs4[:, h:h + 1])
            ps_t = psA.tile([WIN, HEADS, 128], bf16, tag="ptr", name="ps_tr",
                            bufs=1)
            for h in range(HEADS):
                nc.tensor.transpose(ps_t[:, h, :], pn[:, h, :], ident_bf)
            pnT = stream.tile([WIN, HEADS, 128], bf16, tag="pnT", bufs=4)
            nc.gpsimd.tensor_copy(pnT, ps_t)
            ps_o = psA.tile([128, DIM], f32, tag="ov", name="ps_o", bufs=2)
            for h in range(HEADS):
                nc.tensor.matmul(
                    ps_o[:, 128 * h:128 * (h + 1)],
                    v_win[:, t, 128 * h:128 * (h + 1)],
                    pnT[:, h, :],
                    start=True, stop=True)
            nc.gpsimd.tensor_copy(oT[:, 0:2, lt:lt + 128], ps_o[:, 0:256])
            nc.scalar.activation(out=oT[:, 2:4, lt:lt + 128],
                                 in_=ps_o[:, 256:512], func=AF.Copy, scale=1.0)

            # out-proj + LN1 for row tile t (same 128-row block)
            rn = RN[t]
            r0 = 128 * t
            ps_ao = psB.tile([128, DIM], f32, tag="attnout", name="ps_ao")
            for m in range(4):
                nc.tensor.matmul(ps_ao[:rn], oT[:, m, r0:r0 + rn],
                                 owT[:, m, :], start=(m == 0),
                                 stop=(m == 3 and not has_bias))
            if has_bias:
                nc.tensor.matmul(ps_ao[:rn], ones_r[:, :rn], ob_r,
                                 start=False, stop=True)
            stats = stream.tile([128, 6], f32, tag="stats", bufs=6)
            nc.vector.bn_stats(out=stats[:rn], in_=ps_ao[:rn])
            mv = stream.tile([128, 2], f32, tag="mv", bufs=6)
            nc.vector.bn_aggr(out=mv[:rn], in_=stats[:rn])
            lnv = stream.tile([128, 1], f32, tag="lnv", bufs=6)
            nc.scalar.activation(out=lnv[:rn], in_=mv[:rn, 1:2],
                                 func=AF.Ln, bias=eps_t[:rn], scale=1.0)
            rstd = stream.tile([128, 1], f32, tag="rstd", bufs=6)
            nc.scalar.activation(out=rstd[:rn], in_=lnv[:rn],
                                 func=AF.Exp, scale=-0.5)
            nmr = stream.tile([128, 1], f32, tag="nmr", bufs=6)
            nc.vector.tensor_scalar(nmr[:rn], mv[:rn, 0:1], rstd[:rn], -1.0,
                                    ALU.mult, ALU.mult)
            rsh = stream.tile([128, DIM], f32, tag="rsh", bufs=3)
            nc.gpsimd.tensor_scalar_add(rsh[:rn], resid_t[:rn, t, :],
                                        nmr[:rn])
            cnn_rm = stream.tile([128, DIM], bf16, tag="cnn_rm", bufs=4)
            nc.vector.scalar_tensor_tensor(
                out=cnn_rm[:rn], in0=ps_ao[:rn], scalar=rstd[:rn],
                in1=rsh[:rn], op0=ALU.mult, op1=ALU.add)
            ps_ct = psB.tile([128, 4, 128], bf16, tag="cnntr", name="ps_ct",
                             bufs=1)
            for m in range(4):
                nc.tensor.transpose(ps_ct[:, m, :rn],
                                    cnn_rm[:rn, 128 * m:128 * (m + 1)],
                                    ident_bf[:rn, :rn])
            nc.gpsimd.tensor_copy(cnnT[:, :, 1 + r0:1 + r0 + rn],
                                  ps_ct[:, :, :rn])
        psB_cm.__exit__(None, None, None)
        psA_cm.__exit__(None, None, None)
        attnp_cm.__exit__(None, None, None)

        # edge masks on conv input
        nc.gpsimd.tensor_mul(cnnT[:, :, 1:1 + HALO],
                             cnnT[:, :, 1:1 + HALO], edgeL)
        nc.gpsimd.tensor_mul(cnnT[:, :, R + 1 - HALO:R + 1],
                             cnnT[:, :, R + 1 - HALO:R + 1], edgeR)

        # ---- conv stack (bf16) -------------------------------------------
        NTL = [(0, 512), (512, 512), (1024, R - 1024)]
        psC_cm = tc.tile_pool(name="psC", bufs=4, space="PSUM")
        psC = psC_cm.__enter__()
        hs = [cnnT, h1, h2, h3]
        for li in range(NL):
            cur, nxt = hs[li], hs[li + 1]
            for (n0, nn) in NTL:
                for m in range(4):
                    ps = psC.tile([128, 512], f32, tag="conv", name="ps_cv")
                    first = True
                    for d in range(3):
                        for k in range(4):
                            nc.tensor.matmul(
                                ps[:, :nn], cw[li][:, 4 * d + k,
                                                   128 * m:128 * (m + 1)],
                                cur[:, k, n0 + d:n0 + d + nn],
                                start=first, stop=(d == 2 and k == 3))
                            first = False
                    tmp = stream.tile([128, 512], bf16, tag="sp", bufs=4)
                    nc.scalar.activation(
                        out=tmp[:, :nn], in_=ps[:, :nn], func=AF.Exp,
                        bias=cb_t[:, 4 * li + m:4 * li + m + 1], scale=1.0)
                    nc.scalar.activation(
                        out=nxt[:, m, 1 + n0:1 + n0 + nn], in_=tmp[:, :nn],
                        func=AF.Ln, bias=1.0, scale=1.0)
            if li < NL - 1:
                nc.gpsimd.tensor_mul(nxt[:, :, 1:1 + HALO],
                                     nxt[:, :, 1:1 + HALO], edgeL)
                nc.gpsimd.tensor_mul(nxt[:, :, R + 1 - HALO:R + 1],
                                     nxt[:, :, R + 1 - HALO:R + 1], edgeR)

        # ---- skip + h3 + LN2 + output ------------------------------------
        for rt in range(NT):
            rn = RN[rt]
            r0 = 128 * rt
            olo = max(r0, HALO)
            ohi = min(r0 + rn, HALO + RPC)
            if olo >= ohi:
                continue
            ps = psC.tile([128, DIM], f32, tag="skip", name="ps_sk")
            # transposed h3 rows first (first one zeroes the psum bank),
            # then the skip matmuls accumulate on top.
            for m in range(4):
                nc.tensor.matmul(ps[:rn, 128 * m:128 * (m + 1)],
                                 h3[:, m, 1 + r0:1 + r0 + rn],
                                 ident_f, is_transpose=True,
                                 start=(m == 0), stop=False)
            for m in range(4):
                nc.tensor.matmul(ps[:rn], cnnT[:, m, 1 + r0:1 + r0 + rn],
                                 swT[:, m, :], start=False,
                                 stop=(m == 3 and not has_bias))
            if has_bias:
                nc.tensor.matmul(ps[:rn], ones_r[:, :rn], sb_r,
                                 start=False, stop=True)
            stats = stream.tile([128, 6], f32, tag="stats2", bufs=6)
            nc.vector.bn_stats(out=stats[:rn], in_=ps[:rn])
            mv = stream.tile([128, 2], f32, tag="mv2", bufs=6)
            nc.vector.bn_aggr(out=mv[:rn], in_=stats[:rn])
            lnv = stream.tile([128, 1], f32, tag="lnv2", bufs=6)
            nc.scalar.activation(out=lnv[:rn], in_=mv[:rn, 1:2],
                                 func=AF.Ln, bias=eps_t[:rn], scale=1.0)
            rstd = stream.tile([128, 1], f32, tag="rstd2", bufs=6)
            nc.scalar.activation(out=rstd[:rn], in_=lnv[:rn],
                                 func=AF.Exp, scale=-0.5)
            nmr = stream.tile([128, 1], f32, tag="nmr2", bufs=6)
            nc.vector.tensor_scalar(nmr[:rn], mv[:rn, 0:1], rstd[:rn], -1.0,
                                    ALU.mult, ALU.mult)
            out_t = stream.tile([128, DIM], f32, tag="out_t", bufs=3)
            nc.scalar.activation(out=out_t[:rn], in_=ps[:rn],
                                 func=AF.Identity, bias=nmr[:rn],
                                 scale=rstd[:rn])
            nc.sync.dma_start(
                out=out_d[olo - HALO:ohi - HALO, :],
                in_=out_t[olo - r0:ohi - r0, :])
        psC_cm.__exit__(None, None, None)
        ctx.close()
    nc.finalize()
    return nc


def kernel(**inputs):
    from concourse.bass_utils import run_bass_kernel_spmd
    in_maps, has_bias = host_prep(**inputs)
    key = ("nc", has_bias)
    if key not in _CACHE:
        _CACHE[key] = _build_bass(has_bias=has_bias)
        _CACHE["nc"] = _CACHE[key]      # for test.py's TimelineSim hook
    nc = _CACHE[key]
    res = run_bass_kernel_spmd(nc, in_maps, list(range(NC)))
    return np.concatenate([res.results[c]["out"] for c in range(NC)], axis=0)


# revision 17
# speedup vs baseline: 1.0996x; 1.0996x over previous
"""Trainium2 Bass kernel for a sparse-attention decoder block.

Reference computation (single core, jax):
  src = concat([x, pos], 1)                      # [S=2048, 136]
  tgt = (src @ proj_w.T + proj_b) -> [5S, 512]
  q/k/v projections, banded multihead attention (band ~27 src cols/query),
  out-proj, layernorm + residual, 3-layer conv1d (k=3, softplus),
  skip proj, final layernorm.  Output [10240, 512].

Sharding: the 5S=10240 query/row dimension is split across 8 cores
(1280 rows each + 3-row halo each side for the conv stack).  The banded
mask means each core only needs a 304-row slice of src.  Everything else
(weights) is replicated.  No collectives; the conv halo is recomputed
locally; out-of-range halo rows at the sequence edges are zeroed on
device via tiny per-core edge masks (conv zero-padding semantics).

Precision: q/k/v projections run in fp32r (full PE rate at N>=256);
attention (scores, probs, attn-out), out-proj, conv stack and skip-proj
run in bf16 with fp32 PSUM accumulation (measured end-to-end rel err
~3e-3, threshold 2e-2).  bf16 doubles effective PE rate on the small
attention matmuls (which pay a 4x fp32r penalty below 256 cols) and
halves transpose cost and weight DMA traffic.
"""

import numpy as np
import ml_dtypes

S = 2048
PROJ = 128
DIM = 512
NPOS = 8
KD = PROJ + NPOS        # 136
HEADS = 4
HD = DIM // HEADS       # 128
EXT = 64
L = 5 * S               # 10240
NL = 3
NC = 8                  # cores

RPC = L // NC           # 1280 rows per core
HALO = 3
R = RPC + 2 * HALO      # 1286 local rows
W = 304                 # src slice width
WIN = 64                # attention window per 128-query tile
NT = 11                 # query tiles per core
TILE_OFF = [128 * t for t in range(10)] + [R - 128]          # 1158 last
WJ0 = [((lt - 71) // 5) + 16 for lt in TILE_OFF]             # window starts
RN = [min(128, R - 128 * rt) for rt in range(NT)]            # rows per tile

BF16 = ml_dtypes.bfloat16


def _pos_embed():
    n = np.arange(S, dtype=np.float64)[:, None]
    e = np.arange(NPOS)
    return ((n % (2.0 ** (e + 1))) / (2.0 ** e)).astype(np.float32)


def host_prep(x, residual, proj_w, proj_b, q_w, k_w, v_w, in_b, out_w, out_b,
              conv_w, conv_b, skip_w, skip_b):
    """Build the per-core input maps (all numpy, cheap)."""
    x = np.asarray(x, np.float32)
    residual = np.asarray(residual, np.float32)
    proj_w = np.asarray(proj_w, np.float32)
    proj_b = np.asarray(proj_b, np.float32)
    q_w = np.asarray(q_w, np.float32)
    k_w = np.asarray(k_w, np.float32)
    v_w = np.asarray(v_w, np.float32)
    in_b = np.asarray(in_b, np.float32)
    out_w = np.asarray(out_w, np.float32)
    out_b = np.asarray(out_b, np.float32)
    conv_w = np.asarray(conv_w, np.float32)
    conv_b = np.asarray(conv_b, np.float32)
    skip_w = np.asarray(skip_w, np.float32)
    skip_b = np.asarray(skip_b, np.float32)
    src = np.concatenate([x, _pos_embed()], axis=1)            # [S, KD]

    scale = 1.0 / np.sqrt(np.float32(HD))
    # fold proj into q:  q_p = src @ (q_w @ proj_w_p).T + qb_p
    qwT = np.empty((KD, 5 * DIM), np.float32)
    qb = np.empty((5, DIM), np.float32)
    for p in range(5):
        blk = proj_w[DIM * p:DIM * (p + 1), :]                 # [512, KD]
        fused = q_w @ blk                                      # [512, KD]
        qwT[:, DIM * p:DIM * (p + 1)] = fused.T * scale
        qb[p] = (q_w @ proj_b[DIM * p:DIM * (p + 1)] + in_b[:DIM]) * scale
    # k bias dropped (softmax shift invariance); v bias folded into out bias
    out_b_eff = out_b + out_w @ in_b[2 * DIM:3 * DIM]
    has_bias = bool(np.any(out_b_eff) or np.any(skip_b))

    kwT = np.ascontiguousarray(k_w.T).astype(BF16)             # [KD, 512]
    vwT = np.ascontiguousarray(v_w.T)
    qwT_bf = qwT.astype(BF16)

    # out/skip weights: [128, 4, 512] bf16 (chunk m = input features
    # 128m..128m+127 on partitions)
    owT_t = np.ascontiguousarray(
        out_w.T.reshape(4, 128, DIM).transpose(1, 0, 2)).astype(BF16)
    swT_t = np.ascontiguousarray(
        skip_w.T.reshape(4, 128, DIM).transpose(1, 0, 2)).astype(BF16)
    # conv weights: per layer [128, (d,k), 512] bf16:
    #   cw[li][p, 4d+k, j] = conv_w[li, j, 128k+p, d]
    cw_t = np.ascontiguousarray(
        conv_w.transpose(0, 3, 2, 1)                           # [li, d, in, out]
        .reshape(NL, 3, 4, 128, DIM).transpose(0, 3, 1, 2, 4)  # [li,p,d,k,out]
        .reshape(NL, 128, 12, DIM)).astype(BF16)

    qb_t = np.ascontiguousarray(
        qb.reshape(5, 4, 128).transpose(2, 0, 1).reshape(128, 20))
    cb_t = np.ascontiguousarray(
        conv_b.reshape(NL, 4, 128).transpose(2, 0, 1).reshape(128, NL * 4))
    ob_r = (out_b_eff.astype(np.float32)).reshape(1, DIM)
    sb_r = (skip_b.astype(np.float32)).reshape(1, DIM)

    in_maps = []
    for c in range(NC):
        i0c = 256 * c - 16
        gl0 = RPC * c - HALO

        # src slice [W, KD] with zero pad outside [0, S)
        sl = np.zeros((W, KD), np.float32)
        lo, hi = max(0, i0c), min(S, i0c + W)
        sl[lo - i0c:hi - i0c] = src[lo:hi]
        srcT = np.ascontiguousarray(sl.T)                      # [KD, W]

        # residual tiles [128, NT, DIM] (tile rt = local rows TILE_OFF[rt]..)
        rs = np.zeros((R, DIM), np.float32)
        rlo, rhi = max(0, gl0), min(L, gl0 + R)
        rs[rlo - gl0:rhi - gl0] = residual[rlo:rhi]
        resid_t = np.ascontiguousarray(
            np.stack([rs[TILE_OFF[t]:TILE_OFF[t] + 128] for t in range(NT)],
                     axis=1)).astype(BF16)                     # [128, NT, DIM]

        # attention masks, head-replicated: [128, NT*4*WIN] bf16
        m = np.zeros((128, NT, HEADS, WIN), np.float32)
        for t in range(NT):
            gl = gl0 + TILE_OFF[t] + np.arange(128)[:, None]   # [128,1]
            gi = i0c + WJ0[t] + np.arange(WIN)[None, :]        # [1,WIN]
            allowed = ((gi >= 0) & (gi < S) &
                       (gl >= 5 * gi - EXT) & (gl < 5 * gi + 5 + EXT))
            care = (gl >= 0) & (gl < L)                        # real rows
            m[:, t, :, :] = np.where(~care | allowed, 0.0, -30000.0)[:, None, :]
        masks4 = np.ascontiguousarray(m.reshape(128, NT * HEADS * WIN)).astype(BF16)

        edgeL = np.ones((128, 4, HALO), np.float32)
        edgeR = np.ones((128, 4, HALO), np.float32)
        if c == 0:
            edgeL[:] = 0.0
        if c == NC - 1:
            edgeR[:] = 0.0

        in_maps.append({
            "srcT": srcT, "srcT_bf": srcT.astype(BF16),
            "resid": resid_t, "qwT": qwT_bf, "qb": qb_t,
            "kwT": kwT, "vwT": vwT, "owT": owT_t, "swT": swT_t,
            "cw": cw_t, "cb": cb_t, "masks": masks4,
            "edgeL": edgeL.reshape(128, 12), "edgeR": edgeR.reshape(128, 12),
            "ob": ob_r, "sb": sb_r,
        })
    return in_maps, has_bias


# ---------------------------------------------------------------- device ---

_CACHE = {}


def _build_bass(has_bias=False):
    import concourse.bass as bass
    import concourse.mybir as mybir
    import concourse.tile as tile
    from concourse import bacc
    from concourse.masks import make_identity
    from contextlib import ExitStack

    f32 = mybir.dt.float32
    f32r = mybir.dt.float32r
    bf16 = mybir.dt.bfloat16
    AF = mybir.ActivationFunctionType
    ALU = mybir.AluOpType

    # All ACT funcs used here (Exp, Ln, Identity, Copy) live in the single
    # table 'natural_log_exp_and_others'.  The table-load pass picks the
    # first table containing each func, which alternates exp_and_others /
    # natural_log and inserts ~1.3us table loads.  Empty every other
    # table's func set (keeping dict order, so act_func_set_id keeps
    # matching walrus's act_info.json) to pin all activations to the
    # shared table -> one load.  (The toolchain's softplus table maps no
    # Softplus func, so softplus is computed as Ln(1 + Exp(x)).)
    import concourse.hw_specs as _hw
    import concourse.bacc as _bacc_mod
    import concourse.bass_interp as _interp_mod
    if not getattr(_hw, "_act_tables_pinned", False):
        _orig_gat = _hw.get_activation_tables

        def _pinned_gat(arch):
            t = _orig_gat(arch)
            return {name: (funcs if name == "natural_log_exp_and_others"
                           else set())
                    for name, funcs in t.items()}

        _hw.get_activation_tables = _pinned_gat
        _bacc_mod.get_activation_tables = _pinned_gat
        _interp_mod.get_activation_tables = _pinned_gat
        _hw._act_tables_pinned = True

    nc = bacc.Bacc()

    def din(name, shape, dt=f32):
        return nc.dram_tensor(name, shape, dt, kind="ExternalInput")

    srcT_d = din("srcT", [KD, W], f32r)
    srcbf_d = din("srcT_bf", [KD, W], bf16)
    resid_d = din("resid", [128, NT, DIM], bf16)
    qwT_d = din("qwT", [KD, 5 * DIM], bf16)
    qb_d = din("qb", [128, 20])
    kwT_d = din("kwT", [KD, DIM], bf16)
    vwT_d = din("vwT", [KD, DIM], f32r)
    owT_d = din("owT", [128, 4, DIM], bf16)
    swT_d = din("swT", [128, 4, DIM], bf16)
    cw_d = din("cw", [NL, 128, 12, DIM], bf16)
    cb_d = din("cb", [128, NL * 4])
    masks_d = din("masks", [128, NT * HEADS * WIN], bf16)
    edgeL_d = din("edgeL", [128, 12])
    edgeR_d = din("edgeR", [128, 12])
    if has_bias:
        ob_d = din("ob", [1, DIM])
        sb_d = din("sb", [1, DIM])
    out_d = nc.dram_tensor("out", [RPC, DIM], f32, kind="ExternalOutput")

    KCH = [(0, 128), (128, KD - 128)]        # contraction chunks over KD

    ctx = ExitStack()
    with tile.TileContext(nc) as tc:
        persist = ctx.enter_context(tc.tile_pool(name="persist", bufs=1))
        stream = ctx.enter_context(tc.tile_pool(name="stream", bufs=3))

        ident_bf = persist.tile([128, 128], bf16)
        make_identity(nc, ident_bf)
        ident_f = persist.tile([128, 128], f32)
        make_identity(nc, ident_f)
        eps_t = persist.tile([128, 1], f32)
        nc.vector.memset(eps_t, 1e-5)

        # ---- SBUF tensors (phase-scoped pools to fit 208KB) --------------
        attnp_cm = tc.tile_pool(name="attnp", bufs=1)
        attnp = attnp_cm.__enter__()
        qkvw_cm = tc.tile_pool(name="qkvw", bufs=1)
        qkvw = qkvw_cm.__enter__()

        srcT = [qkvw.tile([kn, W], f32r, name=f"srcT{ki}")
                for ki, (k0, kn) in enumerate(KCH)]
        srcB = [qkvw.tile([kn, W], bf16, name=f"srcB{ki}")
                for ki, (k0, kn) in enumerate(KCH)]
        qwT = [qkvw.tile([kn, 5 * DIM], bf16, name=f"qwT{ki}")
               for ki, (k0, kn) in enumerate(KCH)]
        kwT = [qkvw.tile([kn, DIM], bf16, name=f"kwT{ki}")
               for ki, (k0, kn) in enumerate(KCH)]
        vwT = [qkvw.tile([kn, DIM], f32r, name=f"vwT{ki}")
               for ki, (k0, kn) in enumerate(KCH)]
        masks = attnp.tile([128, NT * HEADS * WIN], bf16)
        qb_t = persist.tile([128, 20], f32)
        resid_t = persist.tile([128, NT, DIM], bf16)
        owT = persist.tile([128, 4, DIM], bf16)
        swT = persist.tile([128, 4, DIM], bf16)
        cw = [persist.tile([128, 12, DIM], bf16, name=f"cw{li}")
              for li in range(NL)]
        cb_t = persist.tile([128, NL * 4], f32)
        edgeL = persist.tile([128, 4, HALO], f32)
        edgeR = persist.tile([128, 4, HALO], f32)

        qT = [attnp.tile([128, R + 4], bf16, name=f"qT{m}") for m in range(4)]
        kT = [attnp.tile([128, W], bf16, name=f"kT{m}") for m in range(4)]
        v_win = attnp.tile([WIN, NT, DIM], bf16)
        oT = attnp.tile([128, HEADS, R], bf16)
        cnnT = persist.tile([128, 4, R + 2], bf16)
        h1 = persist.tile([128, 4, R + 2], bf16)
        h2 = persist.tile([128, 4, R + 2], bf16)
        h3 = persist.tile([128, 4, R + 2], f32)

        # ---- input DMAs ---------------------------------------------------
        # All on the SP queue (no compute there; ACT triggers would block
        # the kT/v_win copies behind HWDGE descriptor slots).  The DMA
        # engine is serial, so order by first-use time.
        for ki, (k0, kn) in enumerate(KCH):
            nc.sync.dma_start(out=srcB[ki], in_=srcbf_d[k0:k0 + kn, :])
        for ki, (k0, kn) in enumerate(KCH):
            nc.sync.dma_start(out=kwT[ki], in_=kwT_d[k0:k0 + kn, :])
        for ki, (k0, kn) in enumerate(KCH):
            nc.sync.dma_start(out=srcT[ki], in_=srcT_d[k0:k0 + kn, :])
        for ki, (k0, kn) in enumerate(KCH):
            nc.sync.dma_start(out=vwT[ki], in_=vwT_d[k0:k0 + kn, :])
        nc.sync.dma_start(out=qb_t, in_=qb_d[:, :])
        for ki, (k0, kn) in enumerate(KCH):
            nc.sync.dma_start(out=qwT[ki], in_=qwT_d[k0:k0 + kn, :])
        nc.sync.dma_start(out=masks, in_=masks_d[:, :])
        nc.sync.dma_start(out=owT, in_=owT_d[:, :, :])
        nc.sync.dma_start(out=resid_t[:, 0:3, :], in_=resid_d[:, 0:3, :])
        nc.sync.dma_start(out=resid_t[:, 3:NT, :], in_=resid_d[:, 3:NT, :])
        nc.sync.dma_start(out=cw[0], in_=cw_d[0, :, :, :])
        nc.sync.dma_start(out=cw[1], in_=cw_d[1, :, :, :])
        nc.sync.dma_start(out=cw[2], in_=cw_d[2, :, :, :])
        nc.sync.dma_start(out=swT, in_=swT_d[:, :, :])
        nc.sync.dma_start(out=cb_t, in_=cb_d[:, :])
        nc.sync.dma_start(out=edgeL, in_=edgeL_d[:, :])
        nc.sync.dma_start(out=edgeR, in_=edgeR_d[:, :])
        if has_bias:
            ob_r = persist.tile([1, DIM], f32r)
            sb_r = persist.tile([1, DIM], f32r)
            ones_r = persist.tile([1, 128], f32r)
            nc.scalar.dma_start(out=ob_r, in_=ob_d[:, :])
            nc.scalar.dma_start(out=sb_r, in_=sb_d[:, :])
            nc.vector.memset(ones_r, 1.0)

        # zero the conv pad columns (cols 0 and R+1)
        for tl in (cnnT, h1, h2, h3):
            nc.gpsimd.memset(tl[:, :, 0:1], 0.0)
            nc.gpsimd.memset(tl[:, :, R + 1:R + 2], 0.0)

        # ---- q/k/v projections (fp32r) -----------------------------------
        psP_cm = tc.tile_pool(name="psP", bufs=3, space="PSUM")
        psP = psP_cm.__enter__()
        for m in range(4):
            ps = psP.tile([128, W], f32, tag="proj", name="ps_kproj")
            for ki, (k0, kn) in enumerate(KCH):
                nc.tensor.matmul(ps, kwT[ki][:, 128 * m:128 * (m + 1)],
                                 srcB[ki][:, :],
                                 start=(ki == 0), stop=(ki == len(KCH) - 1))
            nc.scalar.activation(out=kT[m], in_=ps, func=AF.Copy, scale=1.0)

        for t in range(NT):
            ps = psP.tile([WIN, DIM], f32, tag="proj", name="ps_vproj")
            for ki, (k0, kn) in enumerate(KCH):
                nc.tensor.matmul(ps, srcT[ki][:, WJ0[t]:WJ0[t] + WIN],
                                 vwT[ki][:, :],
                                 start=(ki == 0), stop=(ki == len(KCH) - 1))
            nc.scalar.activation(out=v_win[:, t, :], in_=ps, func=AF.Copy,
                                 scale=1.0)

        for p in range(5):
            off = (p + 3) % 5
            cnt = 258                      # padded even (fp32r ISA rule)
            s0 = 16 if p < 2 else 15
            for m in range(4):
                ps = psP.tile([128, cnt], f32, tag="proj", name="ps_qproj")
                for ki, (k0, kn) in enumerate(KCH):
                    nc.tensor.matmul(
                        ps, qwT[ki][:, DIM * p + 128 * m:DIM * p + 128 * (m + 1)],
                        srcB[ki][:, s0:s0 + cnt],
                        start=(ki == 0), stop=(ki == len(KCH) - 1))
                nc.gpsimd.tensor_scalar_add(
                    qT[m][:, off:off + 5 * (cnt - 1) + 1:5], ps,
                    qb_t[:, 4 * p + m:4 * p + m + 1])
        psP_cm.__exit__(None, None, None)
        qkvw_cm.__exit__(None, None, None)

        # ---- attention + out-proj + LN1, interleaved per tile ------------
        psA_cm = tc.tile_pool(name="psA", bufs=2, space="PSUM")
        psA = psA_cm.__enter__()
        psB_cm = tc.tile_pool(name="psB", bufs=2, space="PSUM")
        psB = psB_cm.__enter__()
        for t in range(NT):
            lt = TILE_OFF[t]
            wj = WJ0[t]
            ps_s = psA.tile([128, HEADS * WIN], f32, tag="scores",
                            name="ps_s", bufs=2)
            for h in range(HEADS):
                nc.tensor.matmul(ps_s[:, WIN * h:WIN * (h + 1)],
                                 qT[h][:, lt:lt + 128],
                                 kT[h][:, wj:wj + WIN], start=True, stop=True)
            nc.gpsimd.tensor_add(ps_s, ps_s,
                                 masks[:, 256 * t:256 * (t + 1)])
            probs = stream.tile([128, HEADS, WIN], bf16, tag="probs", bufs=8)
            nc.scalar.activation(out=probs, in_=ps_s, func=AF.Exp)
            sums4 = stream.tile([128, HEADS], f32, tag="sums4", bufs=8)
            nc.vector.tensor_reduce(sums4, probs, axis=mybir.AxisListType.X,
                                    op=ALU.add)
            rs4 = stream.tile([128, HEADS], f32, tag="rs4", bufs=8)
            nc.vector.reciprocal(rs4, sums4)
            pn = stream.tile([128, HEADS, WIN], bf16, tag="pn", bufs=8)
            for h in range(HEADS):
                nc.vector.tensor_scalar_mul(pn[:, h, :], probs[:, h, :],
                                            rs4[:, h:h + 1])
            ps_t = psA.tile([WIN, HEADS, 128], bf16, tag="ptr", name="ps_tr",
                            bufs=1)
            for h in range(HEADS):
                nc.tensor.transpose(ps_t[:, h, :], pn[:, h, :], ident_bf)
            pnT = stream.tile([WIN, HEADS, 128], bf16, tag="pnT", bufs=4)
            nc.gpsimd.tensor_copy(pnT, ps_t)
            ps_o = psA.tile([128, DIM], f32, tag="ov", name="ps_o", bufs=2)
            for h in range(HEADS):
                nc.tensor.matmul(
                    ps_o[:, 128 * h:128 * (h + 1)],
                    v_win[:, t, 128 * h:128 * (h + 1)],
                    pnT[:, h, :],
                    start=True, stop=True)
            nc.gpsimd.tensor_copy(oT[:, 0:2, lt:lt + 128], ps_o[:, 0:256])
            nc.scalar.activation(out=oT[:, 2:4, lt:lt + 128],
                                 in_=ps_o[:, 256:512], func=AF.Copy, scale=1.0)

            # out-proj + LN1 for row tile t (same 128-row block)
            rn = RN[t]
            r0 = 128 * t
            ps_ao = psB.tile([128, DIM], f32, tag="attnout", name="ps_ao")
            for m in range(4):
                nc.tensor.matmul(ps_ao[:rn], oT[:, m, r0:r0 + rn],
                                 owT[:, m, :], start=(m == 0),
                                 stop=(m == 3 and not has_bias))
            if has_bias:
                nc.tensor.matmul(ps_ao[:rn], ones_r[:, :rn], ob_r,
                                 start=False, stop=True)
            stats = stream.tile([128, 6], f32, tag="stats", bufs=6)
            nc.vector.bn_stats(out=stats[:rn], in_=ps_ao[:rn])
            mv = stream.tile([128, 2], f32, tag="mv", bufs=6)
            nc.vector.bn_aggr(out=mv[:rn], in_=stats[:rn])
            lnv = stream.tile([128, 1], f32, tag="lnv", bufs=6)
            nc.scalar.activation(out=lnv[:rn], in_=mv[:rn, 1:2],
                                 func=AF.Ln, bias=eps_t[:rn], scale=1.0)
            rstd = stream.tile([128, 1], f32, tag="rstd", bufs=6)
            nc.scalar.activation(out=rstd[:rn], in_=lnv[:rn],
                                 func=AF.Exp, scale=-0.5)
            nmr = stream.tile([128, 1], f32, tag="nmr", bufs=6)
            nc.vector.tensor_scalar(nmr[:rn], mv[:rn, 0:1], rstd[:rn], -1.0,
                                    ALU.mult, ALU.mult)
            rsh = stream.tile([128, DIM], f32, tag="rsh", bufs=3)
            nc.gpsimd.tensor_scalar_add(rsh[:rn], resid_t[:rn, t, :],
                                        nmr[:rn])
            cnn_rm = stream.tile([128, DIM], bf16, tag="cnn_rm", bufs=4)
            nc.vector.scalar_tensor_tensor(
                out=cnn_rm[:rn], in0=ps_ao[:rn], scalar=rstd[:rn],
                in1=rsh[:rn], op0=ALU.mult, op1=ALU.add)
            ps_ct = psB.tile([128, 4, 128], bf16, tag="cnntr", name="ps_ct",
                             bufs=1)
            for m in range(4):
                nc.tensor.transpose(ps_ct[:, m, :rn],
                                    cnn_rm[:rn, 128 * m:128 * (m + 1)],
                                    ident_bf[:rn, :rn])
            nc.gpsimd.tensor_copy(cnnT[:, :, 1 + r0:1 + r0 + rn],
                                  ps_ct[:, :, :rn])
        psB_cm.__exit__(None, None, None)
        psA_cm.__exit__(None, None, None)
        attnp_cm.__exit__(None, None, None)

        # edge masks on conv input
        nc.gpsimd.tensor_mul(cnnT[:, :, 1:1 + HALO],
                             cnnT[:, :, 1:1 + HALO], edgeL)
        nc.gpsimd.tensor_mul(cnnT[:, :, R + 1 - HALO:R + 1],
                             cnnT[:, :, R + 1 - HALO:R + 1], edgeR)

        # ---- conv stack (bf16) -------------------------------------------
        NTL = [(0, 512), (512, 512), (1024, R - 1024)]
        psC_cm = tc.tile_pool(name="psC", bufs=4, space="PSUM")
        psC = psC_cm.__enter__()
        hs = [cnnT, h1, h2, h3]
        for li in range(NL):
            cur, nxt = hs[li], hs[li + 1]
            for (n0, nn) in NTL:
                for m in range(4):
                    ps = psC.tile([128, 512], f32, tag="conv", name="ps_cv")
                    first = True
                    for d in range(3):
                        for k in range(4):
                            nc.tensor.matmul(
                                ps[:, :nn], cw[li][:, 4 * d + k,
                                                   128 * m:128 * (m + 1)],
                                cur[:, k, n0 + d:n0 + d + nn],
                                start=first, stop=(d == 2 and k == 3))
                            first = False
                    tmp = stream.tile([128, 512], bf16, tag="sp", bufs=4)
                    nc.scalar.activation(
                        out=tmp[:, :nn], in_=ps[:, :nn], func=AF.Exp,
                        bias=cb_t[:, 4 * li + m:4 * li + m + 1], scale=1.0)
                    nc.scalar.activation(
                        out=nxt[:, m, 1 + n0:1 + n0 + nn], in_=tmp[:, :nn],
                        func=AF.Ln, bias=1.0, scale=1.0)
            if li < NL - 1:
                nc.gpsimd.tensor_mul(nxt[:, :, 1:1 + HALO],
                                     nxt[:, :, 1:1 + HALO], edgeL)
                nc.gpsimd.tensor_mul(nxt[:, :, R + 1 - HALO:R + 1],
                                     nxt[:, :, R + 1 - HALO:R + 1], edgeR)

        # ---- skip + h3 + LN2 + output ------------------------------------
        for rt in range(NT):
            rn = RN[rt]
            r0 = 128 * rt
            olo = max(r0, HALO)
            ohi = min(r0 + rn, HALO + RPC)
            if olo >= ohi:
                continue
            ps = psC.tile([128, DIM], f32, tag="skip", name="ps_sk")
            # transposed h3 rows first (first one zeroes the psum bank),
            # then the skip matmuls accumulate on top.
            for m in range(4):
                nc.tensor.matmul(ps[:rn, 128 * m:128 * (m + 1)],
                                 h3[:, m, 1 + r0:1 + r0 + rn],
                                 ident_f, is_transpose=True,
                                 start=(m == 0), stop=False)
            for m in range(4):
                nc.tensor.matmul(ps[:rn], cnnT[:, m, 1 + r0:1 + r0 + rn],
                                 swT[:, m, :], start=False,
                                 stop=(m == 3 and not has_bias))
            if has_bias:
                nc.tensor.matmul(ps[:rn], ones_r[:, :rn], sb_r,
                                 start=False, stop=True)
            stats = stream.tile([128, 6], f32, tag="stats2", bufs=6)
            nc.vector.bn_stats(out=stats[:rn], in_=ps[:rn])
            mv = stream.tile([128, 2], f32, tag="mv2", bufs=6)
            nc.vector.bn_aggr(out=mv[:rn], in_=stats[:rn])
            lnv = stream.tile([128, 1], f32, tag="lnv2", bufs=6)
            nc.scalar.activation(out=lnv[:rn], in_=mv[:rn, 1:2],
                                 func=AF.Ln, bias=eps_t[:rn], scale=1.0)
            rstd = stream.tile([128, 1], f32, tag="rstd2", bufs=6)
            nc.scalar.activation(out=rstd[:rn], in_=lnv[:rn],
                                 func=AF.Exp, scale=-0.5)
            nmr = stream.tile([128, 1], f32, tag="nmr2", bufs=6)
            nc.vector.tensor_scalar(nmr[:rn], mv[:rn, 0:1], rstd[:rn], -1.0,
                                    ALU.mult, ALU.mult)
            out_t = stream.tile([128, DIM], f32, tag="out_t", bufs=3)
            nc.scalar.activation(out=out_t[:rn], in_=ps[:rn],
                                 func=AF.Identity, bias=nmr[:rn],
                                 scale=rstd[:rn])
            nc.sync.dma_start(
                out=out_d[olo - HALO:ohi - HALO, :],
                in_=out_t[olo - r0:ohi - r0, :])
        psC_cm.__exit__(None, None, None)
        ctx.close()
    nc.finalize()
    return nc


def kernel(**inputs):
    from concourse.bass_utils import run_bass_kernel_spmd
    in_maps, has_bias = host_prep(**inputs)
    key = ("nc", has_bias)
    if key not in _CACHE:
        _CACHE[key] = _build_bass(has_bias=has_bias)
        _CACHE["nc"] = _CACHE[key]      # for test.py's TimelineSim hook
    nc = _CACHE[key]
    res = run_bass_kernel_spmd(nc, in_maps, list(range(NC)))
    return np.concatenate([res.results[c]["out"] for c in range(NC)], axis=0)
